# revision 1
# baseline (speedup 1.0000x reference)
"""Trainium2 Bass kernel for nn_NeighbourAggregation (gnn_message_passing).

Full-input contract: kernel(states[4096,8] f32, log_tau scalar f32) -> [4096,12] f32.

Strategy (8 cores, shard the query dim i into 8 slices of 512):
  The reference reduces algebraically to (per query row i):
    dist[i,j] = sqrt(|p_i - p_j|^2 + 1e-8),  W = exp(-dist/tau), W[i,i] = 0
    alpha = W / rowsum(W)
    s1 = alpha @ [pos,vel],  s2 = alpha @ [pos^2,vel^2]
    mu = c_i - s1,  sigma = sqrt(s2 - s1^2 + 1e-6)      (i-offsets cancel)
    group_vel = mean(vel),  vel_dev = vel - group_vel
  On device (per core, tiles laid out [j=128 partitions, i=512 free]):
    - dist^2 via PE matmul with fp16 hi/lo split operands (K=10, fp32-grade
      precision at full PE speed)
    - clamp(max(x,0)+1e-8) on DVE, sqrt on ACT (sqrt table), exp on ACT (exp
      table) with a global +ln(1000) logit shift so W fits fp16 normal range
      (shift cancels in the softmax ratio)
    - diagonal W zeroed via a mask multiply; per-core j-chunks are rotated so
      the diagonal always lands in chunks 0..3 (same NEFF for all cores)
    - moments via PE matmul, W fp16 moving x [Dhi|Dlo] fp16 stationary,
      accumulated fp32 in PSUM
    - finalize: cross-partition moves done with tiny selection matmuls (PE)
      instead of SBUF->SBUF DMA round trips; sigma via one more ACT sqrt;
      PE transposes assemble the [512,12] output
"""

import sys

sys.path.insert(0, "/opt/trn_rl_repo")

import numpy as np

import concourse.bass as bass
import concourse.mybir as mybir
import concourse.tile as tile
from concourse import bacc
from concourse import bass_utils
from concourse.tile_rust import add_dep_helper

F32 = mybir.dt.float32
F16 = mybir.dt.float16
AF = mybir.ActivationFunctionType
ALU = mybir.AluOpType

N = 4096
NCORES = 8
NI = N // NCORES          # 512 queries per core
P = 128                   # partitions
NCHUNK = N // P           # 32 j-chunks
NG = 4                    # big groups of 8 chunks
EXP_SHIFT = float(np.log(1000.0))  # logit shift: W in [~0, 1000], cancels in softmax

_BUILT = None


def _build_bass():
    nc = bacc.Bacc(
        "TRN2",
        target_bir_lowering=False,
        debug=False,
        enable_asserts=False,
    )

    def din(name, shape, dt=F32):
        return nc.dram_tensor(name, shape, dt, kind="ExternalInput").ap()

    statj = din("statj", [10, N], F16)
    movi = din("movi", [10, NI], F16)
    dmom = din("dmom", [P, NCHUNK * 18], F16)
    onescol = din("onescol", [P, 1], F16)
    diagmask = din("diagmask", [P, 4 * NI], F16)
    ct4 = din("ct4", [4, NI])
    ctv = din("ctv", [2, NI])
    actscale = din("actscale", [P, 1])
    actbias = din("actbias", [P, 1])
    biaseps = din("biaseps", [P, 1])
    eps8 = din("eps8", [P, 1])
    ones128 = din("ones128", [1, P])
    ident = din("ident", [4, 4])
    selmerge = din("selmerge", [18, 9])   # [I9; I9]
    sel8 = din("sel8", [9, 8])            # broadcast row 8 -> partitions 0..7
    sel47 = din("sel47", [8, 4])          # select rows 4..7 -> partitions 0..3
    selv23 = din("selv23", [4, 2])        # select rows 2,3 -> partitions 0,1
    out_d = nc.dram_tensor("out", [NI, 12], F32, kind="ExternalOutput").ap()

    with tile.TileContext(nc) as tc:
        with (
            tc.tile_pool(name="consts", bufs=1) as consts,
            tc.tile_pool(name="dist", bufs=NG) as distpool,
            tc.tile_pool(name="d2c", bufs=2) as d2cpool,
            tc.tile_pool(name="w", bufs=3) as wpool,
            tc.tile_pool(name="fin", bufs=1) as fin,
            tc.tile_pool(name="ot", bufs=2) as otpool,
        ):
            # ---- load operands (dist operands first: they gate the start) --
            statj_sb = consts.tile([10, N], F16)
            movi_sb = consts.tile([10, NI], F16)
            dmom_sb = consts.tile([P, NCHUNK * 18], F16)
            onescol_sb = consts.tile([P, 1], F16)
            diagmask_sb = consts.tile([P, 4 * NI], F16)
            ct4_sb = consts.tile([4, NI], F32)
            ctv_sb = consts.tile([2, NI], F32)
            actscale_sb = consts.tile([P, 1], F32)
            actbias_sb = consts.tile([P, 1], F32)
            biaseps_sb = consts.tile([P, 1], F32)
            eps8_sb = consts.tile([P, 1], F32)
            ones128_sb = consts.tile([1, P], F32)
            ident_sb = consts.tile([4, 4], F32)
            selmerge_sb = consts.tile([18, 9], F32)
            sel8_sb = consts.tile([9, 8], F32)
            sel47_sb = consts.tile([8, 4], F32)
            selv23_sb = consts.tile([4, 2], F32)
            for sb, dr in [
                (statj_sb, statj), (movi_sb, movi),
                (actscale_sb, actscale), (actbias_sb, actbias),
                (eps8_sb, eps8),
                (dmom_sb, dmom), (onescol_sb, onescol),
                (diagmask_sb, diagmask), (ct4_sb, ct4), (ctv_sb, ctv),
                (biaseps_sb, biaseps),
                (ones128_sb, ones128), (ident_sb, ident),
                (selmerge_sb, selmerge), (sel8_sb, sel8),
                (sel47_sb, sel47), (selv23_sb, selv23),
            ]:
                nc.sync.dma_start(sb[:], dr[:])

            # trigger the sqrt-table load immediately (no data deps)
            dummy = fin.tile([1, 1], F32, tag="dummy")
            nc.vector.memset(dummy[:], 1.0)
            nc.scalar.activation(dummy[:], dummy[:], AF.Sqrt, bias=0.0)

            # ---- phase A: dist^2 matmuls, sqrt from PSUM (sqrt table),
            # then a DVE max(x,0) pass: sqrt(neg from fp rounding) gives NaN
            # and DVE max(NaN,0)=0, which matches the reference's near-zero
            # distance for such pairs (verified on HW) ------------------------
            dist_tiles = []
            sqrt_insts = []
            with tc.tile_pool(name="psA", bufs=2, space="PSUM") as psA:
              for gi in range(NG):
                draw = d2cpool.tile([P, 4096], F32, tag="draw")
                for half in range(2):
                    ps = psA.tile([P, 2048], F32, tag="psA")
                    for q in range(4):
                        t = gi * 8 + half * 4 + q
                        nc.tensor.matmul(
                            ps[:, q * NI:(q + 1) * NI],
                            lhsT=statj_sb[:, t * P:(t + 1) * P],
                            rhs=movi_sb[:],
                            start=True,
                            stop=True,
                        )
                    si = nc.scalar.activation(
                        draw[:, half * 2048:(half + 1) * 2048],
                        ps[:], AF.Sqrt, bias=eps8_sb[:])
                    sqrt_insts.append(si)
                dist = distpool.tile([P, 4096], F32, tag="dist")
                nc.vector.tensor_scalar(
                    out=dist[:], in0=draw[:],
                    scalar1=0.0, scalar2=None, op0=ALU.max,
                )
                dist_tiles.append(dist)

            # ---- phase B: exp (exp table), diag mask, moment matmuls ----
            psB = tc.tile_pool(name="psB", bufs=1, space="PSUM")
            psBp = psB.__enter__()
            psM = psBp.tile([18, NI], F32, tag="psM")
            psG = psBp.tile([9, 1], F32, tag="psG")
            last_sqrt = sqrt_insts[-1]
            for t in range(NCHUNK):
                nc.tensor.matmul(
                    psG[:],
                    lhsT=dmom_sb[:, t * 18:t * 18 + 9],
                    rhs=onescol_sb[:],
                    start=(t == 0),
                    stop=False,
                )
                nc.tensor.matmul(
                    psG[:],
                    lhsT=dmom_sb[:, t * 18 + 9:t * 18 + 18],
                    rhs=onescol_sb[:],
                    start=False,
                    stop=(t == NCHUNK - 1),
                )
            for gi in range(NG):
                w = wpool.tile([P, 4096], F16, tag="w")
                # split each exp in half so moment matmuls chase at finer grain
                for hh in range(2):
                    ei = nc.scalar.activation(
                        w[:, hh * 2048:(hh + 1) * 2048],
                        dist_tiles[gi][:, hh * 2048:(hh + 1) * 2048], AF.Exp,
                        bias=actbias_sb[:], scale=actscale_sb[:],
                    )
                    # keep ACT phases contiguous: one sqrt<->exp table switch
                    add_dep_helper(ei.ins, last_sqrt.ins, sync=False,
                                   reason="exp after all sqrts (table batch)")
                if gi == 0:
                    # diagonal chunks are rotated to chunks 0..3 on every core
                    nc.vector.tensor_tensor(
                        out=w[:, 0:4 * NI], in0=w[:, 0:4 * NI],
                        in1=diagmask_sb[:], op=ALU.mult,
                    )
                for k in range(8):
                    t = gi * 8 + k
                    nc.tensor.matmul(
                        psM[:],
                        lhsT=dmom_sb[:, t * 18:(t + 1) * 18],
                        rhs=w[:, k * NI:(k + 1) * NI],
                        start=(t == 0),
                        stop=(t == NCHUNK - 1),
                    )

            # ---- finalize ----------------------------------------------
            Mall = fin.tile([18, NI], F32)
            nc.vector.tensor_copy(Mall[:], psM[:])
            gvt = fin.tile([4, 1], F32, tag="gvt")
            nc.vector.tensor_copy(gvt[0:4, :], psG[0:4, :])
            psB.__exit__(None, None, None)

            psFpool = tc.tile_pool(name="psF", bufs=2, space="PSUM")
            psF = psFpool.__enter__()
            psTpool = tc.tile_pool(name="psT", bufs=2, space="PSUM")
            psT = psTpool.__enter__()

            # Msum = Mhi + Mlo via selection matmul (cross-partition add)
            psQ = psF.tile([9, NI], F32, tag="a")
            nc.tensor.matmul(psQ[:], lhsT=selmerge_sb[:], rhs=Mall[:],
                             start=True, stop=True)
            q_sb = fin.tile([9, NI], F32)
            nc.vector.tensor_copy(q_sb[:], psQ[:])
            rinv = fin.tile([9, NI], F32)
            nc.vector.reciprocal_approx_fast(rinv[:], psQ[:])
            # broadcast 1/rowsum (row 8) to partitions 0..7
            psR = psF.tile([8, NI], F32, tag="b")
            nc.tensor.matmul(psR[:], lhsT=sel8_sb[:], rhs=rinv[:],
                             start=True, stop=True)
            s_sb = fin.tile([8, NI], F32)
            nc.vector.tensor_tensor(out=s_sb[:], in0=q_sb[0:8, :], in1=psR[:],
                                    op=ALU.mult)
            # move s2 rows 4..7 down to partitions 0..3
            psS2 = psF.tile([4, NI], F32, tag="a")
            nc.tensor.matmul(psS2[:], lhsT=sel47_sb[:], rhs=s_sb[:],
                             start=True, stop=True)
            t1 = fin.tile([4, NI], F32)
            nc.vector.tensor_tensor(out=t1[:], in0=s_sb[0:4, :],
                                    in1=s_sb[0:4, :], op=ALU.mult)
            sig2 = fin.tile([4, NI], F32)
            nc.vector.tensor_tensor(out=sig2[:], in0=psS2[:], in1=t1[:],
                                    op=ALU.subtract)
            mu_sb = fin.tile([4, NI], F32)
            nc.vector.tensor_tensor(out=mu_sb[:], in0=ct4_sb[:],
                                    in1=s_sb[0:4, :], op=ALU.subtract)
            sigma_sb = fin.tile([4, NI], F32)
            nc.scalar.activation(sigma_sb[:], sig2[:], AF.Sqrt,
                                 bias=biaseps_sb[0:4, :])

            # group_vel: psG rows 2,3 hold mean vx, vy
            psGV = psF.tile([2, 1], F32, tag="b")
            nc.tensor.matmul(psGV[:], lhsT=selv23_sb[:], rhs=gvt[:],
                             start=True, stop=True)
            gv01 = fin.tile([2, 1], F32)
            nc.vector.tensor_copy(gv01[:], psGV[:])
            vd_sb = fin.tile([2, NI], F32)
            nc.vector.tensor_scalar(
                out=vd_sb[:], in0=ctv_sb[:], scalar1=gv01[:], scalar2=None,
                op0=ALU.subtract,
            )
            psGrow = psF.tile([1, 2], F32, tag="a")
            nc.tensor.transpose(psGrow[:], gv01[:], ident_sb[0:2, 0:2])
            growv = fin.tile([1, 2], F32)
            nc.vector.tensor_copy(growv[:], psGrow[:])
            psGB = psF.tile([P, 2], F32, tag="b")
            nc.tensor.matmul(psGB[:], lhsT=ones128_sb[:], rhs=growv[:],
                             start=True, stop=True)

            # ---- transpose + store -------------------------------------
            ot = otpool.tile([P, 48], F32, tag="ot")
            for k in range(4):
                psK = psT.tile([P, 12], F32, tag="psK")
                nc.tensor.transpose(
                    psK[:, 0:4], mu_sb[:, k * P:(k + 1) * P], ident_sb[:]
                )
                nc.tensor.transpose(
                    psK[:, 4:8], sigma_sb[:, k * P:(k + 1) * P], ident_sb[:]
                )
                nc.tensor.transpose(
                    psK[:, 10:12], vd_sb[:, k * P:(k + 1) * P],
                    ident_sb[0:2, 0:2]
                )
                nc.vector.tensor_copy(psK[:, 8:10], psGB[:])
                nc.vector.tensor_copy(ot[:, k * 12:(k + 1) * 12], psK[:])
            out_rr = out_d.rearrange("(k p) d -> p k d", p=P)
            nc.sync.dma_start(out_rr[:], ot[:].rearrange("p (k d) -> p k d", d=12))
            psTpool.__exit__(None, None, None)
            psFpool.__exit__(None, None, None)

    nc.finalize()
    return nc


def _host_prep(states, log_tau):
    states = np.asarray(states, dtype=np.float32)
    tau = np.exp(np.float32(log_tau)).astype(np.float32)
    pos = ((states[:, :2] + states[:, 2:4]) / 2.0).astype(np.float32)
    vel = ((states[:, 4:6] + states[:, 6:8]) / 2.0).astype(np.float32)
    p2 = (pos[:, 0] * pos[:, 0] + pos[:, 1] * pos[:, 1]).astype(np.float32)

    f16 = np.float16
    ph = pos.astype(f16)
    pl = (pos - ph.astype(np.float32)).astype(f16)
    p2h = p2.astype(f16)
    p2l = (p2 - p2h.astype(np.float32)).astype(f16)

    C = np.concatenate([pos, vel], axis=1).astype(np.float32)          # [N,4]
    D = np.concatenate([C, C * C, np.ones((N, 1), np.float32)], 1)     # [N,9]
    Dh = D.astype(f16)
    Dl = (D - Dh.astype(np.float32)).astype(f16)

    ones_n = np.ones(N, f16)
    diagmask = np.ones((P, 4 * NI), f16)
    pp = np.arange(P)
    for k in range(4):
        diagmask[pp, k * NI + P * k + pp] = 0.0

    selmerge = np.concatenate([np.eye(9), np.eye(9)], 0).astype(np.float32)
    sel8 = np.zeros((9, 8), np.float32)
    sel8[8, :] = 1.0
    sel47 = np.zeros((8, 4), np.float32)
    sel47[np.arange(4, 8), np.arange(4)] = 1.0
    selv23 = np.zeros((4, 2), np.float32)
    selv23[np.arange(2, 4), np.arange(2)] = 1.0

    in_maps = []
    for c in range(NCORES):
        # j-chunk rotation: device chunk t holds original chunk (t + 4c) % 32
        jperm = np.concatenate(
            [np.arange(((t + 4 * c) % NCHUNK) * P, ((t + 4 * c) % NCHUNK) * P + P)
             for t in range(NCHUNK)]
        )
        isl = np.arange(NI * c, NI * (c + 1))

        statj_a = np.stack([
            ph[jperm, 0], ph[jperm, 1], pl[jperm, 0], pl[jperm, 1],
            ph[jperm, 0], ph[jperm, 1], p2h[jperm], p2l[jperm],
            ones_n[:N], ones_n[:N],
        ]).astype(f16)                                                 # [10, N]
        m2 = np.float16(-2.0)
        movi_a = np.stack([
            m2 * ph[isl, 0], m2 * ph[isl, 1], m2 * ph[isl, 0], m2 * ph[isl, 1],
            m2 * pl[isl, 0], m2 * pl[isl, 1], ones_n[:NI], ones_n[:NI],
            p2h[isl], p2l[isl],
        ]).astype(f16)                                                 # [10, NI]

        dmom_a = np.empty((P, NCHUNK * 18), f16)
        Dhp = Dh[jperm].reshape(NCHUNK, P, 9)
        Dlp = Dl[jperm].reshape(NCHUNK, P, 9)
        for t in range(NCHUNK):
            dmom_a[:, t * 18:t * 18 + 9] = Dhp[t]
            dmom_a[:, t * 18 + 9:t * 18 + 18] = Dlp[t]

        in_maps.append({
            "statj": statj_a,
            "movi": movi_a,
            "dmom": dmom_a,
            "onescol": np.full((P, 1), 1.0 / N, f16),
            "diagmask": diagmask,
            "ct4": C[isl].T.copy().astype(np.float32),
            "ctv": vel[isl].T.copy().astype(np.float32),
            "actscale": np.full((P, 1), -1.0 / tau, np.float32),
            "actbias": np.full((P, 1), EXP_SHIFT, np.float32),
            "biaseps": np.full((P, 1), 1e-6, np.float32),
            "eps8": np.full((P, 1), 1e-8, np.float32),
            "ones128": np.ones((1, P), np.float32),
            "ident": np.eye(4, dtype=np.float32),
            "selmerge": selmerge,
            "sel8": sel8,
            "sel47": sel47,
            "selv23": selv23,
        })
    return in_maps


def _get_built():
    global _BUILT
    if _BUILT is None:
        _BUILT = _build_bass()
    return _BUILT


def kernel(states, log_tau, _trace=False, _trace_kwargs=None):
    nc = _get_built()
    in_maps = _host_prep(states, log_tau)
    res = bass_utils.run_bass_kernel_spmd(
        nc, in_maps, core_ids=list(range(NCORES)),
        trace=_trace, **(_trace_kwargs or {}),
    )
    out = np.concatenate([res.results[c]["out"] for c in range(NCORES)], axis=0)
    if _trace:
        kernel._last_results = res
    return out.astype(np.float32)



# revision 5
# speedup vs baseline: 1.0780x; 1.0780x over previous
"""Trainium2 Bass kernel for nn_NeighbourAggregation (gnn_message_passing).

Full-input contract: kernel(states[4096,8] f32, log_tau scalar f32) -> [4096,12] f32.

Strategy (8 cores, shard the query dim i into 8 slices of 512):
  Per query row i the reference reduces algebraically to:
    dist[i,j] = sqrt(|p_i - p_j|^2 + eps),  W = exp(-dist/tau + shift), W[i,i] = 0
    s1 = W @ [pos,vel] / rowsum(W),  s2 = W @ [pos^2,vel^2] / rowsum(W)
    mu = c_i - s1,  sigma = sqrt(s2 - s1^2 + 1e-6)      (i-offsets cancel)
    group_vel = mean(vel),  vel_dev = vel - group_vel
  Device schedule per core (tiles laid out [j=128 partitions, i=512 free]):
    - d2 via PE matmul, fp16 hi/lo split operands (K=10), with +3e-5 injected
      through the |p_i|^2 rank-1 term so d2 > 0 always (no NaN clamp pass)
    - dist = sqrt(d2) on ACT straight from PSUM (sqrt table preloaded at t=0)
    - W = exp(-dist/tau + ln(1000)) on ACT (one table switch; shift cancels in
      the softmax ratio and keeps W in fp16 normal range)
    - diagonal W zeroed by a mask multiply on DVE; per-core j-chunks rotated
      so the diagonal lands in chunks 0..3 (same NEFF on all cores)
    - moments via PE matmul, W fp16 x [Dhi|Dlo] fp16, fp32 PSUM accumulation
    - group_vel: DVE reduce over a host-supplied vel^T/N tile (no PE involved)
    - finalize in transposed layout: one merge+transpose matmul per 128-query
      chunk (lhsT = psM columns, rhs = [I9;I9]), then per-partition-scalar DVE
      ops; sigma sqrt reuses the ACT sqrt table reloaded right after the last
      exp (overlaps the moment-matmul tail)
"""

import sys

sys.path.insert(0, "/opt/trn_rl_repo")

import numpy as np

import concourse.bass as bass
import concourse.mybir as mybir
import concourse.tile as tile
from concourse import bacc
from concourse import bass_utils
from concourse.tile_rust import add_dep_helper

F32 = mybir.dt.float32
F16 = mybir.dt.float16
AF = mybir.ActivationFunctionType
ALU = mybir.AluOpType

N = 4096
NCORES = 8
NI = N // NCORES          # 512 queries per core
P = 128                   # partitions
NCHUNK = N // P           # 32 j-chunks
EXP_SHIFT = float(np.log(1000.0))  # logit shift, cancels in softmax
EPS_BIG = 3e-5            # injected into |p_i|^2 so PE-rounded d2 stays > 0

_BUILT = None


def _build_bass():
    nc = bacc.Bacc(
        "TRN2",
        target_bir_lowering=False,
        debug=False,
        enable_asserts=False,
    )

    def din(name, shape, dt=F32):
        return nc.dram_tensor(name, shape, dt, kind="ExternalInput").ap()

    statj = din("statj", [10, N], F16)
    movi = din("movi", [10, NI], F16)
    dmom = din("dmom", [P, NCHUNK * 18], F16)
    diagmask = din("diagmask", [P, 4 * NI], F16)
    velts = din("velts", [2, N])
    cpack = din("cpack", [P, 24])       # ct4t [.,0:16] + ctvt [.,16:24]
    apack = din("apack", [P, 3])        # actscale, actbias, 1e-6
    selmerge = din("selmerge", [18, 9])  # [I9; I9]
    ones128 = din("ones128", [1, P])
    ident2 = din("ident2", [2, 2])
    out_d = nc.dram_tensor("out", [NI, 12], F32, kind="ExternalOutput").ap()

    with tile.TileContext(nc) as tc:
        with (
            tc.tile_pool(name="consts", bufs=1) as consts,
            tc.tile_pool(name="dist", bufs=1) as distpool,
            tc.tile_pool(name="w", bufs=2) as wpool,
            tc.tile_pool(name="fin", bufs=1) as fin,
        ):
            # ---- load operands (statj/movi gate the start) -------------
            statj_sb = consts.tile([10, N], F16)
            movi_sb = consts.tile([10, NI], F16)
            apack_sb = consts.tile([P, 3], F32)
            velts_sb = consts.tile([2, N], F32)
            dmom_sb = consts.tile([P, NCHUNK * 18], F16)
            diagmask_sb = consts.tile([P, 4 * NI], F16)
            cpack_sb = consts.tile([P, 24], F32)
            selmerge_sb = consts.tile([18, 9], F32)
            ones128_sb = consts.tile([1, P], F32)
            ident2_sb = consts.tile([2, 2], F32)
            for sb, dr in [
                (statj_sb, statj), (movi_sb, movi), (apack_sb, apack),
                (velts_sb, velts), (dmom_sb, dmom), (diagmask_sb, diagmask),
                (cpack_sb, cpack), (selmerge_sb, selmerge),
                (ones128_sb, ones128), (ident2_sb, ident2),
            ]:
                nc.sync.dma_start(sb[:], dr[:])

            # trigger the sqrt-table load immediately (no data deps)
            dummy = fin.tile([1, 1], F32, tag="dummy")
            nc.vector.memset(dummy[:], 1.0)
            nc.scalar.activation(dummy[:], dummy[:], AF.Sqrt, bias=0.0)

            # ---- phase A: d2 matmuls -> sqrt from PSUM -----------------
            dist_all = distpool.tile([P, N * 4], F32)   # [128, 16384]
            sqrt_insts = []
            with tc.tile_pool(name="psA", bufs=2, space="PSUM") as psA:
                for h in range(8):
                    ps = psA.tile([P, 2048], F32, tag="psA")
                    for q in range(4):
                        t = h * 4 + q
                        nc.tensor.matmul(
                            ps[:, q * NI:(q + 1) * NI],
                            lhsT=statj_sb[:, t * P:(t + 1) * P],
                            rhs=movi_sb[:],
                            start=True,
                            stop=True,
                        )
                    si = nc.scalar.activation(
                        dist_all[:, h * 2048:(h + 1) * 2048],
                        ps[:], AF.Sqrt, bias=0.0)
                    sqrt_insts.append(si)

            # group_vel on DVE, early (velts is pre-scaled by 1/N)
            gvt = fin.tile([2, 1], F32, tag="gvt")
            nc.vector.tensor_reduce(
                out=gvt[:], in_=velts_sb[:], axis=mybir.AxisListType.X,
                op=ALU.add)

            # ---- phase B: exp (table switch), diag mask, moments -------
            psB = tc.tile_pool(name="psB", bufs=1, space="PSUM")
            psBp = psB.__enter__()
            psM = psBp.tile([18, NI], F32, tag="psM")
            last_sqrt = sqrt_insts[-1]
            w_tiles = [wpool.tile([P, N], F16, tag=f"w{g}", name=f"w{g}")
                       for g in [0, 1]]
            w3 = wpool.tile([P, N], F16, tag="w3")
            mm_t = 0

            def moments(w, k):
                nonlocal mm_t
                nc.tensor.matmul(
                    psM[:],
                    lhsT=dmom_sb[:, mm_t * 18:(mm_t + 1) * 18],
                    rhs=w[:, k * NI:(k + 1) * NI],
                    start=(mm_t == 0),
                    stop=(mm_t == NCHUNK - 1),
                )
                mm_t += 1

            for g in range(3):
                w = w_tiles[g % 2]
                ei = nc.scalar.activation(
                    w[:], dist_all[:, g * N:(g + 1) * N], AF.Exp,
                    bias=apack_sb[:, 1:2], scale=apack_sb[:, 0:1],
                )
                add_dep_helper(ei.ins, last_sqrt.ins, sync=False,
                               reason="exp after all sqrts (table batch)")
                if g == 0:
                    nc.vector.tensor_tensor(
                        out=w[:, 0:4 * NI], in0=w[:, 0:4 * NI],
                        in1=diagmask_sb[:], op=ALU.mult,
                    )
                for k in range(8):
                    moments(w, k)
            # group 3 in quarters to shorten the tail
            last_exp = None
            for qq in range(4):
                ei = nc.scalar.activation(
                    w3[:, qq * 1024:(qq + 1) * 1024],
                    dist_all[:, 3 * N + qq * 1024: 3 * N + (qq + 1) * 1024],
                    AF.Exp, bias=apack_sb[:, 1:2], scale=apack_sb[:, 0:1],
                )
                add_dep_helper(ei.ins, last_sqrt.ins, sync=False,
                               reason="exp after all sqrts (table batch)")
                last_exp = ei
                moments(w3, qq * 2)
                moments(w3, qq * 2 + 1)

            # reload sqrt table right after the last exp (overlaps moment
            # tail + finalize lead-in; sigma sqrt then costs ~0.2us)
            dummy2 = fin.tile([1, 1], F32, tag="dummy2")
            nc.vector.memset(dummy2[:], 1.0)
            s2i = nc.scalar.activation(dummy2[:], dummy2[:], AF.Sqrt, bias=0.0)
            add_dep_helper(s2i.ins, last_exp.ins, sync=False,
                           reason="sqrt table reload after last exp")

            # ---- finalize (transposed layout) --------------------------
            Mall = fin.tile([18, NI], F32)
            nc.vector.tensor_copy(Mall[:], psM[:])
            psB.__exit__(None, None, None)

            psFpool = tc.tile_pool(name="psF", bufs=1, space="PSUM")
            psF = psFpool.__enter__()

            # gv: [2,1] -> [1,2] -> broadcast [128,2]
            psGrow = psF.tile([1, 2], F32, tag="psGrow")
            nc.tensor.transpose(psGrow[:], gvt[:], ident2_sb[:])
            growv = fin.tile([1, 2], F32)
            nc.vector.tensor_copy(growv[:], psGrow[:])
            psGB = psF.tile([P, 2], F32, tag="psGB")
            nc.tensor.matmul(psGB[:], lhsT=ones128_sb[:], rhs=growv[:],
                             start=True, stop=True)
            gvb = fin.tile([P, 2], F32, tag="gvb")
            nc.vector.tensor_copy(gvb[:], psGB[:])

            ot = fin.tile([P, 48], F32, tag="ot")
            sg_all = fin.tile([P, 16], F32, tag="sg")
            ot3 = ot[:].rearrange("p (k d) -> p k d", d=12)
            for k in range(4):
                psT = psF.tile([P, 9], F32, tag=f"psT{k}")
                nc.tensor.matmul(psT[:], lhsT=Mall[:, k * P:(k + 1) * P],
                                 rhs=selmerge_sb[:], start=True, stop=True)
                rinv = fin.tile([P, 1], F32, tag=f"rinv{k}")
                nc.vector.reciprocal_approx_fast(rinv[:], psT[:, 8:9])
                s_k = fin.tile([P, 8], F32, tag=f"s{k}")
                nc.vector.tensor_scalar(
                    out=s_k[:], in0=psT[:, 0:8], scalar1=rinv[:],
                    scalar2=None, op0=ALU.mult)
                # mu = c - s1  (Pool)
                nc.gpsimd.tensor_tensor(
                    out=ot3[:, k, 0:4], in0=cpack_sb[:, 4 * k:4 * k + 4],
                    in1=s_k[:, 0:4], op=ALU.subtract)
                # sig2 = s2 - s1^2  (DVE)
                t2 = fin.tile([P, 4], F32, tag=f"t2{k}")
                nc.vector.tensor_tensor(out=t2[:], in0=s_k[:, 0:4],
                                        in1=s_k[:, 0:4], op=ALU.mult)
                nc.vector.tensor_tensor(out=sg_all[:, 4 * k:4 * k + 4],
                                        in0=s_k[:, 4:8], in1=t2[:],
                                        op=ALU.subtract)
                # vel_dev + group_vel columns (Pool)
                nc.gpsimd.tensor_tensor(
                    out=ot3[:, k, 10:12],
                    in0=cpack_sb[:, 16 + 2 * k:16 + 2 * k + 2],
                    in1=gvb[:], op=ALU.subtract)
                nc.gpsimd.tensor_copy(ot3[:, k, 8:10], gvb[:])

            # sigma for all 4 chunks in one strided ACT sqrt
            nc.scalar.activation(
                ot3[:, :, 4:8],
                sg_all[:].rearrange("p (k d) -> p k d", d=4),
                AF.Sqrt, bias=apack_sb[:, 2:3])

            out_rr = out_d.rearrange("(k p) d -> p k d", p=P)
            nc.sync.dma_start(out_rr[:], ot3[:])
            psFpool.__exit__(None, None, None)

    nc.finalize()
    return nc


def _host_prep(states, log_tau):
    states = np.asarray(states, dtype=np.float32)
    tau = np.exp(np.float32(log_tau)).astype(np.float32)
    pos = ((states[:, :2] + states[:, 2:4]) / 2.0).astype(np.float32)
    vel = ((states[:, 4:6] + states[:, 6:8]) / 2.0).astype(np.float32)
    p2 = (pos[:, 0] * pos[:, 0] + pos[:, 1] * pos[:, 1]).astype(np.float32)
    p2i = (p2 + np.float32(EPS_BIG)).astype(np.float32)

    f16 = np.float16
    ph = pos.astype(f16)
    pl = (pos - ph.astype(np.float32)).astype(f16)
    p2h = p2.astype(f16)
    p2l = (p2 - p2h.astype(np.float32)).astype(f16)
    p2ih = p2i.astype(f16)
    p2il = (p2i - p2ih.astype(np.float32)).astype(f16)

    C = np.concatenate([pos, vel], axis=1).astype(np.float32)          # [N,4]
    D = np.concatenate([C, C * C, np.ones((N, 1), np.float32)], 1)     # [N,9]
    Dh = D.astype(f16)
    Dl = (D - Dh.astype(np.float32)).astype(f16)

    ones_n = np.ones(N, f16)
    diagmask = np.ones((P, 4 * NI), f16)
    pp = np.arange(P)
    for k in range(4):
        diagmask[pp, k * NI + P * k + pp] = 0.0

    selmerge = np.concatenate([np.eye(9), np.eye(9)], 0).astype(np.float32)
    velts = (vel.T / np.float32(N)).copy().astype(np.float32)          # [2,N]

    in_maps = []
    for c in range(NCORES):
        # j-chunk rotation: device chunk t holds original chunk (t + 4c) % 32
        jperm = np.concatenate(
            [np.arange(((t + 4 * c) % NCHUNK) * P, ((t + 4 * c) % NCHUNK) * P + P)
             for t in range(NCHUNK)]
        )
        isl = np.arange(NI * c, NI * (c + 1))

        statj_a = np.stack([
            ph[jperm, 0], ph[jperm, 1], pl[jperm, 0], pl[jperm, 1],
            ph[jperm, 0], ph[jperm, 1], p2h[jperm], p2l[jperm],
            ones_n[:N], ones_n[:N],
        ]).astype(f16)                                                 # [10, N]
        m2 = np.float16(-2.0)
        movi_a = np.stack([
            m2 * ph[isl, 0], m2 * ph[isl, 1], m2 * ph[isl, 0], m2 * ph[isl, 1],
            m2 * pl[isl, 0], m2 * pl[isl, 1], ones_n[:NI], ones_n[:NI],
            p2ih[isl], p2il[isl],
        ]).astype(f16)                                                 # [10, NI]

        dmom_a = np.empty((P, NCHUNK * 18), f16)
        Dhp = Dh[jperm].reshape(NCHUNK, P, 9)
        Dlp = Dl[jperm].reshape(NCHUNK, P, 9)
        for t in range(NCHUNK):
            dmom_a[:, t * 18:t * 18 + 9] = Dhp[t]
            dmom_a[:, t * 18 + 9:t * 18 + 18] = Dlp[t]

        # transposed per-chunk constants: [128, 16] C and [128, 8] vel
        ct4t = C[isl].reshape(4, P, 4).transpose(1, 0, 2).reshape(P, 16)
        ctvt = vel[isl].reshape(4, P, 2).transpose(1, 0, 2).reshape(P, 8)
        cpack = np.concatenate([ct4t, ctvt], axis=1).astype(np.float32)

        apack = np.stack([
            np.full(P, -1.0 / tau, np.float32),
            np.full(P, EXP_SHIFT, np.float32),
            np.full(P, 1e-6, np.float32),
        ], axis=1)

        in_maps.append({
            "statj": statj_a,
            "movi": movi_a,
            "dmom": dmom_a,
            "diagmask": diagmask,
            "velts": velts,
            "cpack": cpack,
            "apack": apack,
            "selmerge": selmerge,
            "ones128": np.ones((1, P), np.float32),
            "ident2": np.eye(2, dtype=np.float32),
        })
    return in_maps


def _get_built():
    global _BUILT
    if _BUILT is None:
        _BUILT = _build_bass()
    return _BUILT


def kernel(states, log_tau, _trace=False, _trace_kwargs=None):
    nc = _get_built()
    in_maps = _host_prep(states, log_tau)
    res = bass_utils.run_bass_kernel_spmd(
        nc, in_maps, core_ids=list(range(NCORES)),
        trace=_trace, **(_trace_kwargs or {}),
    )
    out = np.concatenate([res.results[c]["out"] for c in range(NCORES)], axis=0)
    if _trace:
        kernel._last_results = res
    return out.astype(np.float32)
